# revision 6
# baseline (speedup 1.0000x reference)
"""Multi-head attention (b=4, n=2048, dim=512, heads=8, d_head=64) on 8 TRN2 NeuronCores.

Sharding: core = 2*b + head_group (data parallel over batch, tensor parallel
over 2 groups of 4 heads). Each core: QKV projection for its 4 heads, full
attention, partial output projection; host sums the 2 partials per batch.

v2 device algorithm (per core), improvements over v1:
  - Scores matmuls are issued as adjacent (row-group 0, row-group 64) pairs:
    head 2hp lives in partitions 0:64 of the qkT tiles, head 2hp+1 in 64:128,
    so the two K=64 matmuls run CONCURRENTLY in disjoint PE row groups
    (measured 117 ns/MM vs 215 serial).  HW erratum: a K=64 rg0 matmul and a
    K=64 rg64 matmul must never hit the same PSUM bank back-to-back -> each
    pexp bank is statically assigned one row group (even bank = rg0 head,
    odd bank = rg64 head).
  - exp is split between ScalarE (exact ACT) and VectorE (one-instruction
    Schraudolph approximation: int16(round(a*S + b)) bitcast to bf16,
    ~1.8% rms) on a fixed schedule, so the softmax is no longer
    single-engine-bound.
  - softmax denominators: reciprocal_approx_fast (single DVE op) and a single
    [64,512] normalize multiply per (head, i-block).
  - Per-(jt) pipeline: scores pair -> exp (N=1024 covers both heads) ->
    PV pair, with PSUM double buffering; projections drip into PE gaps.
"""

import functools
import sys

if "/opt/trn_rl_repo" not in sys.path:
    sys.path.insert(0, "/opt/trn_rl_repo")

import numpy as np
import ml_dtypes

import concourse.bacc as bacc
import concourse.mybir as mybir
import concourse.tile as tile
from concourse.bass_utils import run_bass_kernel_spmd

N_CORES = 8
B = 4
N = 2048          # sequence length
C = 512           # model dim
HPC = 4           # heads per core
D = 64            # head dim
SCALE = D ** -0.5

F32 = mybir.dt.float32
BF16 = mybir.dt.bfloat16
I16 = mybir.dt.int16

NT = N // 128     # 16 j tiles of 128
KT = C // 128     # 4 contraction tiles for the projections
IB = 4            # i-blocks of 512

# Schraudolph approx exp -> bf16 bits: I16 = round(A*x + B), bitcast bf16.
LOG2E = 1.4426950408889634
SCHR_A = 128.0 * LOG2E * SCALE   # scale folded in
SCHR_B = 128.0 * 127.0 - 7.5

# steps (jt) within each unit routed to DVE approx exp (f = 5/16 ~ 0.31)
DVE_STEPS = frozenset({2, 5, 8, 11, 14})

# feature flags for A/B testing (within-process, same device)
# Measured (same session, baseline-calibrated): shared_pool costs ~25us
# (projection chunks stall the scores slot rotation), gden costs ~20us
# (gpsimd 1-partition copies are slow).
_BASE = dict(stage_split=True, tail_split=True, warm_first=True,
             shared_pool=False, gden=False, superstep=False,
             lag=6, ppv_bufs=2, pproj_bufs=2, f50=False)
_BASE["n_warm"] = 14
_BASE["dve_steps"] = frozenset({2, 5, 8, 11, 14})
_BASE["dma_spread"] = True   # spreading input DMAs over 3 queues: -5us
CONFIGS = {
    "cur": dict(_BASE),
    "dnarrow": dict(_BASE, dma_spread=False),
    "v3": dict(_BASE, superstep=True),
    "lag5": dict(_BASE, lag=5),
    "pv31": dict(_BASE, ppv_bufs=3, pproj_bufs=1),
    "f50": dict(_BASE, f50=True),
    # steer DVE exp steps away from the norm-chain DVE work (steps ~5-8)
    "dvepos": dict(_BASE, dve_steps=frozenset({1, 3, 10, 12, 14})),
    "f25": dict(_BASE, dve_steps=frozenset({2, 10, 12, 14})),
}


def _build_body(nc, tc, ctx, xT_d, wqkT_d, wvT_d, woT_d, out_d, cfg):
    sb = ctx.enter_context(tc.tile_pool(name="sb", bufs=1))
    work = ctx.enter_context(tc.tile_pool(name="work", bufs=6))
    ptp = ctx.enter_context(tc.tile_pool(name="ptp", bufs=1))
    if cfg["shared_pool"]:
        # pexp serves scores/exp AND the projection chains (tag-shared
        # slots): 3 x [128,1024] = 6 banks + 2 PV accumulator banks = all 8.
        # Bank parity stays fixed per row group (scores h_even -> even bank,
        # h_odd -> odd bank); K=128 projection writes are safe on any bank.
        pexp = ctx.enter_context(tc.tile_pool(name="pexp", bufs=3, space="PSUM"))
        ppv = ctx.enter_context(tc.tile_pool(name="ppv", bufs=2, space="PSUM"))
        pproj = pexp
    else:
        pexp = ctx.enter_context(tc.tile_pool(name="pexp", bufs=2, space="PSUM"))
        ppv = ctx.enter_context(
            tc.tile_pool(name="ppv", bufs=cfg["ppv_bufs"], space="PSUM"))
        pproj = ctx.enter_context(
            tc.tile_pool(name="pproj", bufs=cfg["pproj_bufs"], space="PSUM"))

    exp_t = mybir.ActivationFunctionType.Exp
    mult_t = mybir.AluOpType.mult
    add_t = mybir.AluOpType.add
    PROJ_W = 1024 if cfg["shared_pool"] else 512
    PROJ_TAG = "ps" if cfg["shared_pool"] else "pp"

    # ---- persistent SBUF tensors ----
    xT = [sb.tile([128, N], BF16, tag=f"x{k}", name=f"x{k}") for k in range(KT)]
    wqk = [sb.tile([128, 512], BF16, tag=f"wqk{k}", name=f"wqk{k}") for k in range(KT)]
    wv = [sb.tile([128, 256], BF16, tag=f"wv{k}", name=f"wv{k}") for k in range(KT)]
    wo = [sb.tile([128, 512], BF16, tag=f"wo{t}", name=f"wo{t}") for t in range(2)]
    qkT = [sb.tile([128, N], BF16, tag=f"qk{o}", name=f"qk{o}") for o in range(4)]
    vsb = [sb.tile([128, HPC * 65], BF16, tag=f"v{t}", name=f"v{t}") for t in range(NT)]
    AT = [sb.tile([128, N], BF16, tag=f"at{t}", name=f"at{t}") for t in range(2)]

    # ---- input DMAs, ordered by first use and spread over 4 engine queues
    # so the first qk-projection inputs land as early as possible ----
    dq = [nc.sync, nc.scalar, nc.gpsimd, nc.gpsimd]
    if not cfg["dma_spread"]:
        dq = [nc.sync, nc.scalar, nc.sync, nc.scalar]
    for k in range(KT):
        dq[k % 4].dma_start(out=wqk[k][:], in_=wqkT_d[k * 128:(k + 1) * 128, :])
    for k in range(KT):
        dq[(k + 2) % 4].dma_start(
            out=xT[k][:, 0:512], in_=xT_d[k * 128:(k + 1) * 128, 0:512])
    for nch in range(1, 4):
        for k in range(KT):
            dq[k % 2].dma_start(
                out=xT[k][:, nch * 512:(nch + 1) * 512],
                in_=xT_d[k * 128:(k + 1) * 128, nch * 512:(nch + 1) * 512],
            )
    for k in range(KT):
        dq[2 + k % 2].dma_start(out=wv[k][:], in_=wvT_d[k * 128:(k + 1) * 128, :])
    for t in range(2):
        dq[2 + t].dma_start(out=wo[t][:], in_=woT_d[t * 128:(t + 1) * 128, :])

    # HAM warm-up: ~4us of dummy matmuls while the input DMAs stream, so the
    # first real matmuls run at 2.4 GHz instead of the cold 1.2 GHz default.
    # The dummy memset comes FIRST on the DVE queue so the matmuls can start
    # ~0.3us in.
    def emit_warmup():
        dummy = sb.tile([128, 512], BF16, tag="dummy", name="dummy")
        nc.vector.memset(dummy[:], 1.0)
        dps = pproj.tile([128, PROJ_W], F32, tag=PROJ_TAG, name="warm")
        for i in range(cfg["n_warm"]):
            nc.tensor.matmul(dps[:, 0:512], dummy[:, 0:128], dummy[:],
                             start=True, stop=True)

    if cfg["warm_first"]:
        emit_warmup()
    # ones columns of v tiles (never overwritten by the v eviction)
    for t in range(NT):
        v3 = vsb[t][:].rearrange("p (h c) -> p h c", c=65)
        nc.vector.memset(v3[:, :, 64:65], 1.0)
    if not cfg["warm_first"]:
        emit_warmup()

    # ---- projection emitters (dripped into the attention pipeline) ----
    # qk o-tiles: 0 = q heads 0/1, 1 = q heads 2/3, 2 = k heads 0/1, 3 = k 2/3.
    def qk_chunk(ot, nch):
        def f():
            ps = pproj.tile([128, PROJ_W], F32, tag=PROJ_TAG, name="pp")
            for k in range(KT):
                nc.tensor.matmul(
                    ps[:, 0:512],
                    wqk[k][:, ot * 128:(ot + 1) * 128],
                    xT[k][:, nch * 512:(nch + 1) * 512],
                    start=(k == 0),
                    stop=(k == KT - 1),
                )
            nc.scalar.copy(qkT[ot][:, nch * 512:(nch + 1) * 512], ps[:, 0:512])
        return f

    def v_chunk(t):
        def f():
            ps = pproj.tile([128, PROJ_W], F32, tag=PROJ_TAG, name="pp")
            for k in range(KT):
                nc.tensor.matmul(
                    ps[:, 0:256],
                    xT[k][:, t * 128:(t + 1) * 128],
                    wv[k][:],
                    start=(k == 0),
                    stop=(k == KT - 1),
                )
            v3 = vsb[t][:].rearrange("p (h c) -> p h c", c=65)
            p3 = ps[:, 0:256].rearrange("p (h c) -> p h c", c=64)
            nc.vector.tensor_copy(v3[:, :, 0:64], p3)
        return f

    def proj_chunk(nt):
        def f():
            pp = pproj.tile([128, PROJ_W], F32, tag=PROJ_TAG, name="pp")
            for t2 in range(2):
                nc.tensor.matmul(
                    pp[:, 0:512],
                    AT[t2][:, nt * 128:(nt + 1) * 128],
                    wo[t2][:],
                    start=(t2 == 0),
                    stop=(t2 == 1),
                )
            ot_s = work.tile([128, 512], BF16, tag="o", name="ot_s")
            nc.vector.tensor_copy(ot_s[:], pp[:, 0:512])
            nc.sync.dma_start(out=out_d[nt * 128:(nt + 1) * 128, :], in_=ot_s[:])
        return f

    def emit_pv(hp, jt, pt, pu_e, pu_o):
        nc.tensor.matmul(
            pu_e[0:65, :], vsb[jt][:, (2 * hp) * 65:(2 * hp) * 65 + 65],
            pt[:, 0:512], start=(jt == 0), stop=(jt == NT - 1))
        nc.tensor.matmul(
            pu_o[0:65, :], vsb[jt][:, (2 * hp + 1) * 65:(2 * hp + 1) * 65 + 65],
            pt[:, 512:1024], start=(jt == 0), stop=(jt == NT - 1))

    def stage_u(h, pu):
        # Stage [U | den] to SBUF: releases the PSUM accumulator for the next
        # unit's PV chain, and gives reciprocal_approx_fast the SBUF,
        # partition-0 input it requires (its raw-bit seed is wrong on the
        # PSUM read path and at nonzero base partitions). In the superstep
        # config these copies ride ScalarE (DVE carries half the exp stream).
        usb = work.tile([65, 512], F32, tag=f"usb{h % 2}", name="usb")
        den = work.tile([1, 512], F32, tag=f"den{h % 2}", name="den")
        if cfg["superstep"]:
            nc.scalar.copy(usb[:], pu[0:65, :])
            nc.scalar.copy(den[:], usb[64:65, :])
        else:
            nc.vector.tensor_copy(usb[:], pu[0:65, :])
            nc.vector.tensor_copy(den[:], usb[64:65, :])
        return usb, den

    def norm_math(h, ib, usb, den, c0=0, c1=512):
        cs = slice(c0, c1)
        rc = work.tile([1, 512], F32, tag=f"rc{h % 2}", name="rc")
        nc.vector.reciprocal_approx_fast(out=rc[0:1, cs], in_=den[0:1, cs])
        rbc = work.tile([64, 512], F32, tag=f"rbc{h % 2}", name="rbc")
        nc.gpsimd.partition_broadcast(rbc[:, cs], rc[0:1, cs], channels=64)
        rows = slice((h % 2) * 64, (h % 2) * 64 + 64)
        nc.vector.tensor_tensor(
            out=AT[h // 2][rows, ib * 512 + c0:ib * 512 + c1],
            in0=usb[0:64, cs], in1=rbc[:, cs], op=mult_t)

    # ---- pipeline ----
    # Prologue: k/q tiles for heads 0/1 (chunk 0 asap), then drip the rest.
    qk_chunk(2, 0)()
    qk_chunk(0, 0)()
    # Filler order contract: unit 0 pops fillers[s] right before step s
    # (20 pops with the PV drain), so v_chunk(jt) must sit at index <= jt+4
    # (PV(jt) is emitted after the pop at step jt+LAG), and qk_chunk(2, c)
    # at index <= 4*c - 1 (scores step 4c reads it after the pop at 4c).
    fillers = [qk_chunk(2, 1), qk_chunk(2, 2), qk_chunk(2, 3), qk_chunk(0, 1)]
    fillers += [v_chunk(t) for t in range(NT)]
    fillers += [qk_chunk(0, 2), qk_chunk(0, 3)]
    fillers += [qk_chunk(1, 0), qk_chunk(1, 1), qk_chunk(1, 2), qk_chunk(1, 3)]
    fillers += [qk_chunk(3, 0), qk_chunk(3, 1), qk_chunk(3, 2), qk_chunk(3, 3)]

    # Global software pipeline across all 8 units: scores/exp for step g run
    # alongside PV for step g-LAG (possibly of the previous unit), so the PE
    # never drains at unit boundaries and HAM stays warm.
    units = [(hp, ib) for hp in range(2) for ib in range(IB)]
    LAG = cfg["lag"]
    total = len(units) * NT
    pts = {}
    pu_e = pu_o = None
    pss = {}

    def emit_scores(g):
        u, s = g // NT, g % NT
        hp, ib = units[u]
        q_t = qkT[hp]
        k_t = qkT[2 + hp]
        # scores pair: concurrent row groups, fixed bank parity
        ps = pexp.tile([128, 1024], F32, tag="ps", name="ps")
        nc.tensor.matmul(
            ps[:, 0:512], k_t[0:64, s * 128:(s + 1) * 128],
            q_t[0:64, ib * 512:(ib + 1) * 512], start=True, stop=True)
        nc.tensor.matmul(
            ps[:, 512:1024], k_t[64:128, s * 128:(s + 1) * 128],
            q_t[64:128, ib * 512:(ib + 1) * 512], start=True, stop=True)
        pss[g] = ps

    def emit_exp(g, on_dve):
        ps = pss.pop(g)
        pt = ptp.tile([128, 1024], BF16, tag=f"pt{g % 8}", name=f"pt{g % 8}")
        if on_dve:
            nc.vector.tensor_scalar(
                out=pt[:].bitcast(I16), in0=ps[:],
                scalar1=SCHR_A, scalar2=SCHR_B, op0=mult_t, op1=add_t)
        else:
            nc.scalar.activation(pt[:], ps[:], exp_t, scale=SCALE)
        pts[g] = pt

    def handle_pv(pg):
        nonlocal pu_e, pu_o
        u2, s2 = pg // NT, pg % NT
        hp2, ib2 = units[u2]
        if s2 == 0:
            pu_e = ppv.tile([128, 512], F32, tag="pu", name="pue")
            pu_o = ppv.tile([128, 512], F32, tag="pu", name="puo")
        emit_pv(hp2, s2, pts.pop(pg), pu_e, pu_o)
        if s2 == NT - 1:
            if cfg["tail_split"] and u2 == len(units) - 1:
                # Kernel tail: nothing reuses these PSUM banks, so skip the
                # SBUF staging, normalize straight out of PSUM in 128-col
                # quarters, and release each quarter's output-projection
                # chunk asap. The two den copies run on different engines.
                de = work.tile([1, 512], F32, tag="den0", name="den")
                nc.vector.tensor_copy(de[:], pu_e[64:65, :])
                do_ = work.tile([1, 512], F32, tag="den1", name="den")
                nc.scalar.copy(do_[:], pu_o[64:65, :])
                for c in range(4):
                    norm_math(2 * hp2, ib2, pu_e, de, c * 128, (c + 1) * 128)
                    norm_math(2 * hp2 + 1, ib2, pu_o, do_, c * 128, (c + 1) * 128)
                    fillers.append(proj_chunk(ib2 * 4 + c))
            else:
                # release both PSUM accumulators first, then the math
                ue, de = stage_u(2 * hp2, pu_e)
                uo, do_ = stage_u(2 * hp2 + 1, pu_o)
                if hp2 == 1:
                    # interleave 256-col norm halves with their dependent
                    # output-projection chunks so they start ~1.3us earlier
                    for c in range(2):
                        norm_math(2 * hp2, ib2, ue, de, c * 256, (c + 1) * 256)
                        norm_math(2 * hp2 + 1, ib2, uo, do_,
                                  c * 256, (c + 1) * 256)
                        fillers.append(proj_chunk(ib2 * 4 + 2 * c))
                        fillers.append(proj_chunk(ib2 * 4 + 2 * c + 1))
                else:
                    norm_math(2 * hp2, ib2, ue, de)
                    norm_math(2 * hp2 + 1, ib2, uo, do_)

    if cfg["superstep"]:
        # two steps per iteration: 4 score MMs back-to-back (the rg0/rg64
        # pairs of consecutive steps overlap; the K=128 drain bubble is paid
        # once), exp(even)->ScalarE and exp(odd)->DVE run concurrently, then
        # the two lagged PV pairs.
        for G in range(0, total + LAG, 2):
            for g in (G, G + 1):
                if g < total:
                    if fillers:
                        fillers.pop(0)()
                    emit_scores(g)
            for g in (G, G + 1):
                if g < total:
                    emit_exp(g, on_dve=(g % 2 == 1))
            for g in (G - LAG, G + 1 - LAG):
                if 0 <= g < total:
                    handle_pv(g)
    else:
        for g in range(total + LAG):
            if g < total:
                u, s = g // NT, g % NT
                if fillers:
                    fillers.pop(0)()
                emit_scores(g)
                if cfg["f50"]:
                    on_dve = (s % 2 == 1)
                elif cfg["tail_split"] and u == len(units) - 1 and s >= 8:
                    on_dve = (s % 2 == 1)
                else:
                    on_dve = s in cfg["dve_steps"]
                emit_exp(g, on_dve)
            if g - LAG >= 0:
                handle_pv(g - LAG)
    for f in fillers:
        f()


@functools.lru_cache(maxsize=8)
def _build(variant="cur"):
    nc = bacc.Bacc("TRN2", target_bir_lowering=False, debug=False,
                   num_devices=N_CORES)
    xT_d = nc.dram_tensor("xT", [C, N], BF16, kind="ExternalInput").ap()
    wqkT_d = nc.dram_tensor("wqkT", [C, 512], BF16, kind="ExternalInput").ap()
    wvT_d = nc.dram_tensor("wvT", [C, 256], BF16, kind="ExternalInput").ap()
    woT_d = nc.dram_tensor("woT", [256, C], BF16, kind="ExternalInput").ap()
    out_d = nc.dram_tensor("out", [N, C], BF16, kind="ExternalOutput").ap()
    from contextlib import ExitStack
    with tile.TileContext(nc) as tc, ExitStack() as ctx:
        _build_body(nc, tc, ctx, xT_d, wqkT_d, wvT_d, woT_d, out_d,
                    CONFIGS[variant])
    nc.compile()
    return nc


def _shard_inputs(x, W_qkv, W_out):
    bf16 = ml_dtypes.bfloat16
    in_maps = []
    for core in range(N_CORES):
        b, hg = core // 2, core % 2
        xT = np.ascontiguousarray(x[b].T).astype(bf16)
        rows_q = W_qkv[hg * 256:(hg + 1) * 256, :]
        rows_k = W_qkv[512 + hg * 256:512 + (hg + 1) * 256, :]
        wqkT = np.ascontiguousarray(
            np.concatenate([rows_q, rows_k], 0).T).astype(bf16)
        wvT = np.ascontiguousarray(
            W_qkv[1024 + hg * 256:1024 + (hg + 1) * 256, :].T).astype(bf16)
        woT = np.ascontiguousarray(
            W_out[:, hg * 256:(hg + 1) * 256].T).astype(bf16)
        in_maps.append({"xT": xT, "wqkT": wqkT, "wvT": wvT, "woT": woT})
    return in_maps


def _run(inputs, trace=False, tmpdir=None, variant="cur"):
    x = np.asarray(inputs["x"], dtype=np.float32)
    W_qkv = np.asarray(inputs["W_qkv"], dtype=np.float32)
    W_out = np.asarray(inputs["W_out"], dtype=np.float32)
    nc = _build(variant)
    in_maps = _shard_inputs(x, W_qkv, W_out)
    kwargs = {}
    if trace:
        kwargs = dict(trace=True, tmpdir=tmpdir)
    res = run_bass_kernel_spmd(nc, in_maps, core_ids=list(range(N_CORES)), **kwargs)
    out = np.zeros((B, N, C), np.float32)
    for core in range(N_CORES):
        out[core // 2] += res.results[core]["out"].astype(np.float32)
    return out, res


def kernel(**inputs):
    out, _ = _run(inputs)
    return out


# revision 7
# speedup vs baseline: 1.0099x; 1.0099x over previous
"""Multi-head attention (b=4, n=2048, dim=512, heads=8, d_head=64) on 8 TRN2 NeuronCores.

Sharding: core = 2*b + head_group (data parallel over batch, tensor parallel
over 2 groups of 4 heads). Each core: QKV projection for its 4 heads, full
attention, partial output projection; host sums the 2 partials per batch.

v2 device algorithm (per core), improvements over v1:
  - Scores matmuls are issued as adjacent (row-group 0, row-group 64) pairs:
    head 2hp lives in partitions 0:64 of the qkT tiles, head 2hp+1 in 64:128,
    so the two K=64 matmuls run CONCURRENTLY in disjoint PE row groups
    (measured 117 ns/MM vs 215 serial).  HW erratum: a K=64 rg0 matmul and a
    K=64 rg64 matmul must never hit the same PSUM bank back-to-back -> each
    pexp bank is statically assigned one row group (even bank = rg0 head,
    odd bank = rg64 head).
  - exp is split between ScalarE (exact ACT) and VectorE (one-instruction
    Schraudolph approximation: int16(round(a*S + b)) bitcast to bf16,
    ~1.8% rms) on a fixed schedule, so the softmax is no longer
    single-engine-bound.
  - softmax denominators: reciprocal_approx_fast (single DVE op) and a single
    [64,512] normalize multiply per (head, i-block).
  - Per-(jt) pipeline: scores pair -> exp (N=1024 covers both heads) ->
    PV pair, with PSUM double buffering; projections drip into PE gaps.
"""

import functools
import sys

if "/opt/trn_rl_repo" not in sys.path:
    sys.path.insert(0, "/opt/trn_rl_repo")

import numpy as np
import ml_dtypes

import concourse.bacc as bacc
import concourse.mybir as mybir
import concourse.tile as tile
from concourse.bass_utils import run_bass_kernel_spmd

N_CORES = 8
B = 4
N = 2048          # sequence length
C = 512           # model dim
HPC = 4           # heads per core
D = 64            # head dim
SCALE = D ** -0.5

F32 = mybir.dt.float32
BF16 = mybir.dt.bfloat16
I16 = mybir.dt.int16

NT = N // 128     # 16 j tiles of 128
KT = C // 128     # 4 contraction tiles for the projections
IB = 4            # i-blocks of 512

# Schraudolph approx exp -> bf16 bits: I16 = round(A*x + B), bitcast bf16.
LOG2E = 1.4426950408889634
SCHR_A = 128.0 * LOG2E * SCALE   # scale folded in
SCHR_B = 128.0 * 127.0 - 7.5

# steps (jt) within each unit routed to DVE approx exp (f = 5/16 ~ 0.31)
DVE_STEPS = frozenset({2, 5, 8, 11, 14})

# feature flags for A/B testing (within-process, same device)
# Measured (same session, baseline-calibrated): shared_pool costs ~25us
# (projection chunks stall the scores slot rotation), gden costs ~20us
# (gpsimd 1-partition copies are slow).
_BASE = dict(stage_split=True, tail_split=True, warm_first=True,
             shared_pool=False, gden=False, superstep=False,
             lag=6, ppv_bufs=2, pproj_bufs=2, f50=False)
_BASE["n_warm"] = 14
_BASE["dve_steps"] = frozenset({2, 5, 8, 11, 14})
_BASE["dma_spread"] = True   # spreading input DMAs over 3 queues: -5us
CONFIGS = {
    "cur": dict(_BASE),
    "dnarrow": dict(_BASE, dma_spread=False),
    "v3": dict(_BASE, superstep=True),
    "lag5": dict(_BASE, lag=5),
    "pv31": dict(_BASE, ppv_bufs=3, pproj_bufs=1),
    "f50": dict(_BASE, f50=True),
    # steer DVE exp steps away from the norm-chain DVE work (steps ~5-8)
    "dvepos": dict(_BASE, dve_steps=frozenset({1, 3, 10, 12, 14})),
    "f25": dict(_BASE, dve_steps=frozenset({2, 10, 12, 14})),
}


def _build_body(nc, tc, ctx, xT_d, wqkT_d, wvT_d, woT_d, out_d, cfg):
    sb = ctx.enter_context(tc.tile_pool(name="sb", bufs=1))
    work = ctx.enter_context(tc.tile_pool(name="work", bufs=6))
    ptp = ctx.enter_context(tc.tile_pool(name="ptp", bufs=1))
    if cfg["shared_pool"]:
        # pexp serves scores/exp AND the projection chains (tag-shared
        # slots): 3 x [128,1024] = 6 banks + 2 PV accumulator banks = all 8.
        # Bank parity stays fixed per row group (scores h_even -> even bank,
        # h_odd -> odd bank); K=128 projection writes are safe on any bank.
        pexp = ctx.enter_context(tc.tile_pool(name="pexp", bufs=3, space="PSUM"))
        ppv = ctx.enter_context(tc.tile_pool(name="ppv", bufs=2, space="PSUM"))
        pproj = pexp
    else:
        pexp = ctx.enter_context(tc.tile_pool(name="pexp", bufs=2, space="PSUM"))
        ppv = ctx.enter_context(
            tc.tile_pool(name="ppv", bufs=cfg["ppv_bufs"], space="PSUM"))
        pproj = ctx.enter_context(
            tc.tile_pool(name="pproj", bufs=cfg["pproj_bufs"], space="PSUM"))

    exp_t = mybir.ActivationFunctionType.Exp
    mult_t = mybir.AluOpType.mult
    add_t = mybir.AluOpType.add
    PROJ_W = 1024 if cfg["shared_pool"] else 512
    PROJ_TAG = "ps" if cfg["shared_pool"] else "pp"

    # ---- persistent SBUF tensors ----
    xT = [sb.tile([128, N], BF16, tag=f"x{k}", name=f"x{k}") for k in range(KT)]
    wqk = [sb.tile([128, 512], BF16, tag=f"wqk{k}", name=f"wqk{k}") for k in range(KT)]
    wv = [sb.tile([128, 256], BF16, tag=f"wv{k}", name=f"wv{k}") for k in range(KT)]
    wo = [sb.tile([128, 512], BF16, tag=f"wo{t}", name=f"wo{t}") for t in range(2)]
    qkT = [sb.tile([128, N], BF16, tag=f"qk{o}", name=f"qk{o}") for o in range(4)]
    vsb = [sb.tile([128, HPC * 65], BF16, tag=f"v{t}", name=f"v{t}") for t in range(NT)]
    AT = [sb.tile([128, N], BF16, tag=f"at{t}", name=f"at{t}") for t in range(2)]

    # ---- input DMAs, ordered by first use and spread over 4 engine queues
    # so the first qk-projection inputs land as early as possible ----
    dq = [nc.sync, nc.scalar, nc.gpsimd, nc.gpsimd]
    if not cfg["dma_spread"]:
        dq = [nc.sync, nc.scalar, nc.sync, nc.scalar]
    for k in range(KT):
        dq[k % 4].dma_start(out=wqk[k][:], in_=wqkT_d[k * 128:(k + 1) * 128, :])
    for k in range(KT):
        dq[(k + 2) % 4].dma_start(
            out=xT[k][:, 0:512], in_=xT_d[k * 128:(k + 1) * 128, 0:512])
    for nch in range(1, 4):
        for k in range(KT):
            dq[k % 2].dma_start(
                out=xT[k][:, nch * 512:(nch + 1) * 512],
                in_=xT_d[k * 128:(k + 1) * 128, nch * 512:(nch + 1) * 512],
            )
    for k in range(KT):
        dq[2 + k % 2].dma_start(out=wv[k][:], in_=wvT_d[k * 128:(k + 1) * 128, :])
    for t in range(2):
        dq[2 + t].dma_start(out=wo[t][:], in_=woT_d[t * 128:(t + 1) * 128, :])

    # HAM warm-up: ~4us of dummy matmuls while the input DMAs stream, so the
    # first real matmuls run at 2.4 GHz instead of the cold 1.2 GHz default.
    # The dummy memset comes FIRST on the DVE queue so the matmuls can start
    # ~0.3us in.
    def emit_warmup():
        dummy = sb.tile([128, 512], BF16, tag="dummy", name="dummy")
        nc.vector.memset(dummy[:], 1.0)
        dps = pproj.tile([128, PROJ_W], F32, tag=PROJ_TAG, name="warm")
        for i in range(cfg["n_warm"]):
            nc.tensor.matmul(dps[:, 0:512], dummy[:, 0:128], dummy[:],
                             start=True, stop=True)

    if cfg["warm_first"]:
        emit_warmup()
    # ones columns of v tiles (never overwritten by the v eviction)
    for t in range(NT):
        v3 = vsb[t][:].rearrange("p (h c) -> p h c", c=65)
        nc.vector.memset(v3[:, :, 64:65], 1.0)
    if not cfg["warm_first"]:
        emit_warmup()

    # ---- projection emitters (dripped into the attention pipeline) ----
    # qk o-tiles: 0 = q heads 0/1, 1 = q heads 2/3, 2 = k heads 0/1, 3 = k 2/3.
    def qk_chunk(ot, nch):
        def f():
            ps = pproj.tile([128, PROJ_W], F32, tag=PROJ_TAG, name="pp")
            for k in range(KT):
                nc.tensor.matmul(
                    ps[:, 0:512],
                    wqk[k][:, ot * 128:(ot + 1) * 128],
                    xT[k][:, nch * 512:(nch + 1) * 512],
                    start=(k == 0),
                    stop=(k == KT - 1),
                )
            nc.scalar.copy(qkT[ot][:, nch * 512:(nch + 1) * 512], ps[:, 0:512])
        return f

    def v_chunk(t):
        def f():
            ps = pproj.tile([128, PROJ_W], F32, tag=PROJ_TAG, name="pp")
            for k in range(KT):
                nc.tensor.matmul(
                    ps[:, 0:256],
                    xT[k][:, t * 128:(t + 1) * 128],
                    wv[k][:],
                    start=(k == 0),
                    stop=(k == KT - 1),
                )
            v3 = vsb[t][:].rearrange("p (h c) -> p h c", c=65)
            p3 = ps[:, 0:256].rearrange("p (h c) -> p h c", c=64)
            nc.vector.tensor_copy(v3[:, :, 0:64], p3)
        return f

    def proj_chunk(nt, evict_scalar=False):
        def f():
            pp = pproj.tile([128, PROJ_W], F32, tag=PROJ_TAG, name="pp")
            for t2 in range(2):
                nc.tensor.matmul(
                    pp[:, 0:512],
                    AT[t2][:, nt * 128:(nt + 1) * 128],
                    wo[t2][:],
                    start=(t2 == 0),
                    stop=(t2 == 1),
                )
            ot_s = work.tile([128, 512], BF16, tag="o", name="ot_s")
            if evict_scalar:
                # kernel tail: ScalarE is idle; keep DVE free for the norms
                nc.scalar.copy(ot_s[:], pp[:, 0:512])
            else:
                nc.vector.tensor_copy(ot_s[:], pp[:, 0:512])
            nc.sync.dma_start(out=out_d[nt * 128:(nt + 1) * 128, :], in_=ot_s[:])
        return f

    def emit_pv(hp, jt, pt, pu_e, pu_o):
        nc.tensor.matmul(
            pu_e[0:65, :], vsb[jt][:, (2 * hp) * 65:(2 * hp) * 65 + 65],
            pt[:, 0:512], start=(jt == 0), stop=(jt == NT - 1))
        nc.tensor.matmul(
            pu_o[0:65, :], vsb[jt][:, (2 * hp + 1) * 65:(2 * hp + 1) * 65 + 65],
            pt[:, 512:1024], start=(jt == 0), stop=(jt == NT - 1))

    def stage_u(h, pu):
        # Stage [U | den] to SBUF: releases the PSUM accumulator for the next
        # unit's PV chain, and gives reciprocal_approx_fast the SBUF,
        # partition-0 input it requires (its raw-bit seed is wrong on the
        # PSUM read path and at nonzero base partitions). In the superstep
        # config these copies ride ScalarE (DVE carries half the exp stream).
        usb = work.tile([65, 512], F32, tag=f"usb{h % 2}", name="usb")
        den = work.tile([1, 512], F32, tag=f"den{h % 2}", name="den")
        if cfg["superstep"]:
            nc.scalar.copy(usb[:], pu[0:65, :])
            nc.scalar.copy(den[:], usb[64:65, :])
        else:
            nc.vector.tensor_copy(usb[:], pu[0:65, :])
            nc.vector.tensor_copy(den[:], usb[64:65, :])
        return usb, den

    def norm_math(h, ib, usb, den, c0=0, c1=512):
        cs = slice(c0, c1)
        rc = work.tile([1, 512], F32, tag=f"rc{h % 2}", name="rc")
        nc.vector.reciprocal_approx_fast(out=rc[0:1, cs], in_=den[0:1, cs])
        rbc = work.tile([64, 512], F32, tag=f"rbc{h % 2}", name="rbc")
        nc.gpsimd.partition_broadcast(rbc[:, cs], rc[0:1, cs], channels=64)
        rows = slice((h % 2) * 64, (h % 2) * 64 + 64)
        nc.vector.tensor_tensor(
            out=AT[h // 2][rows, ib * 512 + c0:ib * 512 + c1],
            in0=usb[0:64, cs], in1=rbc[:, cs], op=mult_t)

    # ---- pipeline ----
    # Prologue: k/q tiles for heads 0/1 (chunk 0 asap), then drip the rest.
    qk_chunk(2, 0)()
    qk_chunk(0, 0)()
    # Filler order contract: unit 0 pops fillers[s] right before step s
    # (20 pops with the PV drain), so v_chunk(jt) must sit at index <= jt+4
    # (PV(jt) is emitted after the pop at step jt+LAG), and qk_chunk(2, c)
    # at index <= 4*c - 1 (scores step 4c reads it after the pop at 4c).
    fillers = [qk_chunk(2, 1), qk_chunk(2, 2), qk_chunk(2, 3), qk_chunk(0, 1)]
    fillers += [v_chunk(t) for t in range(NT)]
    fillers += [qk_chunk(0, 2), qk_chunk(0, 3)]
    fillers += [qk_chunk(1, 0), qk_chunk(1, 1), qk_chunk(1, 2), qk_chunk(1, 3)]
    fillers += [qk_chunk(3, 0), qk_chunk(3, 1), qk_chunk(3, 2), qk_chunk(3, 3)]

    # Global software pipeline across all 8 units: scores/exp for step g run
    # alongside PV for step g-LAG (possibly of the previous unit), so the PE
    # never drains at unit boundaries and HAM stays warm.
    units = [(hp, ib) for hp in range(2) for ib in range(IB)]
    LAG = cfg["lag"]
    total = len(units) * NT
    pts = {}
    pu_e = pu_o = None
    pss = {}

    def emit_scores(g):
        u, s = g // NT, g % NT
        hp, ib = units[u]
        q_t = qkT[hp]
        k_t = qkT[2 + hp]
        # scores pair: concurrent row groups, fixed bank parity
        ps = pexp.tile([128, 1024], F32, tag="ps", name="ps")
        nc.tensor.matmul(
            ps[:, 0:512], k_t[0:64, s * 128:(s + 1) * 128],
            q_t[0:64, ib * 512:(ib + 1) * 512], start=True, stop=True)
        nc.tensor.matmul(
            ps[:, 512:1024], k_t[64:128, s * 128:(s + 1) * 128],
            q_t[64:128, ib * 512:(ib + 1) * 512], start=True, stop=True)
        pss[g] = ps

    def emit_exp(g, on_dve):
        ps = pss.pop(g)
        pt = ptp.tile([128, 1024], BF16, tag=f"pt{g % 8}", name=f"pt{g % 8}")
        if on_dve:
            nc.vector.tensor_scalar(
                out=pt[:].bitcast(I16), in0=ps[:],
                scalar1=SCHR_A, scalar2=SCHR_B, op0=mult_t, op1=add_t)
        else:
            nc.scalar.activation(pt[:], ps[:], exp_t, scale=SCALE)
        pts[g] = pt

    def handle_pv(pg):
        nonlocal pu_e, pu_o
        u2, s2 = pg // NT, pg % NT
        hp2, ib2 = units[u2]
        if s2 == 0:
            pu_e = ppv.tile([128, 512], F32, tag="pu", name="pue")
            pu_o = ppv.tile([128, 512], F32, tag="pu", name="puo")
        emit_pv(hp2, s2, pts.pop(pg), pu_e, pu_o)
        if s2 == NT - 1:
            if cfg["tail_split"] and u2 == len(units) - 1:
                # Kernel tail: nothing reuses these PSUM banks, so skip the
                # SBUF staging, normalize straight out of PSUM in 128-col
                # quarters, and release each quarter's output-projection
                # chunk asap. The two den copies run on different engines.
                de = work.tile([1, 512], F32, tag="den0", name="den")
                nc.vector.tensor_copy(de[:], pu_e[64:65, :])
                do_ = work.tile([1, 512], F32, tag="den1", name="den")
                nc.scalar.copy(do_[:], pu_o[64:65, :])
                for c in range(4):
                    norm_math(2 * hp2, ib2, pu_e, de, c * 128, (c + 1) * 128)
                    norm_math(2 * hp2 + 1, ib2, pu_o, do_, c * 128, (c + 1) * 128)
                    fillers.append(proj_chunk(ib2 * 4 + c, evict_scalar=True))
            else:
                # release both PSUM accumulators first, then the math
                ue, de = stage_u(2 * hp2, pu_e)
                uo, do_ = stage_u(2 * hp2 + 1, pu_o)
                if hp2 == 1:
                    # interleave 256-col norm halves with their dependent
                    # output-projection chunks so they start ~1.3us earlier
                    for c in range(2):
                        norm_math(2 * hp2, ib2, ue, de, c * 256, (c + 1) * 256)
                        norm_math(2 * hp2 + 1, ib2, uo, do_,
                                  c * 256, (c + 1) * 256)
                        fillers.append(proj_chunk(ib2 * 4 + 2 * c))
                        fillers.append(proj_chunk(ib2 * 4 + 2 * c + 1))
                else:
                    norm_math(2 * hp2, ib2, ue, de)
                    norm_math(2 * hp2 + 1, ib2, uo, do_)

    if cfg["superstep"]:
        # two steps per iteration: 4 score MMs back-to-back (the rg0/rg64
        # pairs of consecutive steps overlap; the K=128 drain bubble is paid
        # once), exp(even)->ScalarE and exp(odd)->DVE run concurrently, then
        # the two lagged PV pairs.
        for G in range(0, total + LAG, 2):
            for g in (G, G + 1):
                if g < total:
                    if fillers:
                        fillers.pop(0)()
                    emit_scores(g)
            for g in (G, G + 1):
                if g < total:
                    emit_exp(g, on_dve=(g % 2 == 1))
            for g in (G - LAG, G + 1 - LAG):
                if 0 <= g < total:
                    handle_pv(g)
    else:
        for g in range(total + LAG):
            if g < total:
                u, s = g // NT, g % NT
                if fillers:
                    fillers.pop(0)()
                emit_scores(g)
                if cfg["f50"]:
                    on_dve = (s % 2 == 1)
                elif cfg["tail_split"] and u == len(units) - 1 and s >= 8:
                    on_dve = (s % 2 == 1)
                else:
                    on_dve = s in cfg["dve_steps"]
                emit_exp(g, on_dve)
            if g - LAG >= 0:
                handle_pv(g - LAG)
    for f in fillers:
        f()


@functools.lru_cache(maxsize=8)
def _build(variant="cur"):
    nc = bacc.Bacc("TRN2", target_bir_lowering=False, debug=False,
                   num_devices=N_CORES)
    xT_d = nc.dram_tensor("xT", [C, N], BF16, kind="ExternalInput").ap()
    wqkT_d = nc.dram_tensor("wqkT", [C, 512], BF16, kind="ExternalInput").ap()
    wvT_d = nc.dram_tensor("wvT", [C, 256], BF16, kind="ExternalInput").ap()
    woT_d = nc.dram_tensor("woT", [256, C], BF16, kind="ExternalInput").ap()
    out_d = nc.dram_tensor("out", [N, C], BF16, kind="ExternalOutput").ap()
    from contextlib import ExitStack
    with tile.TileContext(nc) as tc, ExitStack() as ctx:
        _build_body(nc, tc, ctx, xT_d, wqkT_d, wvT_d, woT_d, out_d,
                    CONFIGS[variant])
    nc.compile()
    return nc


def _shard_inputs(x, W_qkv, W_out):
    bf16 = ml_dtypes.bfloat16
    in_maps = []
    for core in range(N_CORES):
        b, hg = core // 2, core % 2
        xT = np.ascontiguousarray(x[b].T).astype(bf16)
        rows_q = W_qkv[hg * 256:(hg + 1) * 256, :]
        rows_k = W_qkv[512 + hg * 256:512 + (hg + 1) * 256, :]
        wqkT = np.ascontiguousarray(
            np.concatenate([rows_q, rows_k], 0).T).astype(bf16)
        wvT = np.ascontiguousarray(
            W_qkv[1024 + hg * 256:1024 + (hg + 1) * 256, :].T).astype(bf16)
        woT = np.ascontiguousarray(
            W_out[:, hg * 256:(hg + 1) * 256].T).astype(bf16)
        in_maps.append({"xT": xT, "wqkT": wqkT, "wvT": wvT, "woT": woT})
    return in_maps


def _run(inputs, trace=False, tmpdir=None, variant="cur"):
    x = np.asarray(inputs["x"], dtype=np.float32)
    W_qkv = np.asarray(inputs["W_qkv"], dtype=np.float32)
    W_out = np.asarray(inputs["W_out"], dtype=np.float32)
    nc = _build(variant)
    in_maps = _shard_inputs(x, W_qkv, W_out)
    kwargs = {}
    if trace:
        kwargs = dict(trace=True, tmpdir=tmpdir)
    res = run_bass_kernel_spmd(nc, in_maps, core_ids=list(range(N_CORES)), **kwargs)
    out = np.zeros((B, N, C), np.float32)
    for core in range(N_CORES):
        out[core // 2] += res.results[core]["out"].astype(np.float32)
    return out, res


def kernel(**inputs):
    out, _ = _run(inputs)
    return out


# revision 8
# speedup vs baseline: 1.0162x; 1.0063x over previous
"""Multi-head attention (b=4, n=2048, dim=512, heads=8, d_head=64) on 8 TRN2 NeuronCores.

Sharding: core = 2*b + head_group (data parallel over batch, tensor parallel
over 2 groups of 4 heads). Each core: QKV projection for its 4 heads, full
attention, partial output projection; host sums the 2 partials per batch.

v2 device algorithm (per core), improvements over v1:
  - Scores matmuls are issued as adjacent (row-group 0, row-group 64) pairs:
    head 2hp lives in partitions 0:64 of the qkT tiles, head 2hp+1 in 64:128,
    so the two K=64 matmuls run CONCURRENTLY in disjoint PE row groups
    (measured 117 ns/MM vs 215 serial).  HW erratum: a K=64 rg0 matmul and a
    K=64 rg64 matmul must never hit the same PSUM bank back-to-back -> each
    pexp bank is statically assigned one row group (even bank = rg0 head,
    odd bank = rg64 head).
  - exp is split between ScalarE (exact ACT) and VectorE (one-instruction
    Schraudolph approximation: int16(round(a*S + b)) bitcast to bf16,
    ~1.8% rms) on a fixed schedule, so the softmax is no longer
    single-engine-bound.
  - softmax denominators: reciprocal_approx_fast (single DVE op) and a single
    [64,512] normalize multiply per (head, i-block).
  - Per-(jt) pipeline: scores pair -> exp (N=1024 covers both heads) ->
    PV pair, with PSUM double buffering; projections drip into PE gaps.
"""

import functools
import sys

if "/opt/trn_rl_repo" not in sys.path:
    sys.path.insert(0, "/opt/trn_rl_repo")

import numpy as np
import ml_dtypes

import concourse.bacc as bacc
import concourse.mybir as mybir
import concourse.tile as tile
from concourse.bass_utils import run_bass_kernel_spmd

N_CORES = 8
B = 4
N = 2048          # sequence length
C = 512           # model dim
HPC = 4           # heads per core
D = 64            # head dim
SCALE = D ** -0.5

F32 = mybir.dt.float32
BF16 = mybir.dt.bfloat16
I16 = mybir.dt.int16

NT = N // 128     # 16 j tiles of 128
KT = C // 128     # 4 contraction tiles for the projections
IB = 4            # i-blocks of 512

# Schraudolph approx exp -> bf16 bits: I16 = round(A*x + B), bitcast bf16.
LOG2E = 1.4426950408889634
SCHR_A = 128.0 * LOG2E * SCALE   # scale folded in
SCHR_B = 128.0 * 127.0 - 7.5

# steps (jt) within each unit routed to DVE approx exp (f = 5/16 ~ 0.31)
DVE_STEPS = frozenset({2, 5, 8, 11, 14})

# feature flags for A/B testing (within-process, same device)
# Measured (same session, baseline-calibrated): shared_pool costs ~25us
# (projection chunks stall the scores slot rotation), gden costs ~20us
# (gpsimd 1-partition copies are slow).
_BASE = dict(stage_split=True, tail_split=True, warm_first=True,
             shared_pool=False, gden=False, superstep=False,
             lag=6, ppv_bufs=2, pproj_bufs=2, f50=False)
_BASE["n_warm"] = 14
_BASE["dve_steps"] = frozenset({2, 5, 8, 11, 14})
_BASE["dma_spread"] = True   # spreading input DMAs over 3 queues: -5us
_BASE["sevict"] = False
CONFIGS = {
    "cur": dict(_BASE),
    "sevict": dict(_BASE, sevict=True),
    "dnarrow": dict(_BASE, dma_spread=False),
    "v3": dict(_BASE, superstep=True),
    "lag5": dict(_BASE, lag=5),
    "pv31": dict(_BASE, ppv_bufs=3, pproj_bufs=1),
    "f50": dict(_BASE, f50=True),
    # steer DVE exp steps away from the norm-chain DVE work (steps ~5-8)
    "dvepos": dict(_BASE, dve_steps=frozenset({1, 3, 10, 12, 14})),
    "f25": dict(_BASE, dve_steps=frozenset({2, 10, 12, 14})),
}


def _build_body(nc, tc, ctx, xT_d, wqkT_d, wvT_d, woT_d, out_d, cfg):
    sb = ctx.enter_context(tc.tile_pool(name="sb", bufs=1))
    work = ctx.enter_context(tc.tile_pool(name="work", bufs=6))
    ptp = ctx.enter_context(tc.tile_pool(name="ptp", bufs=1))
    if cfg["shared_pool"]:
        # pexp serves scores/exp AND the projection chains (tag-shared
        # slots): 3 x [128,1024] = 6 banks + 2 PV accumulator banks = all 8.
        # Bank parity stays fixed per row group (scores h_even -> even bank,
        # h_odd -> odd bank); K=128 projection writes are safe on any bank.
        pexp = ctx.enter_context(tc.tile_pool(name="pexp", bufs=3, space="PSUM"))
        ppv = ctx.enter_context(tc.tile_pool(name="ppv", bufs=2, space="PSUM"))
        pproj = pexp
    else:
        pexp = ctx.enter_context(tc.tile_pool(name="pexp", bufs=2, space="PSUM"))
        ppv = ctx.enter_context(
            tc.tile_pool(name="ppv", bufs=cfg["ppv_bufs"], space="PSUM"))
        pproj = ctx.enter_context(
            tc.tile_pool(name="pproj", bufs=cfg["pproj_bufs"], space="PSUM"))

    exp_t = mybir.ActivationFunctionType.Exp
    mult_t = mybir.AluOpType.mult
    add_t = mybir.AluOpType.add
    PROJ_W = 1024 if cfg["shared_pool"] else 512
    PROJ_TAG = "ps" if cfg["shared_pool"] else "pp"

    # ---- persistent SBUF tensors ----
    xT = [sb.tile([128, N], BF16, tag=f"x{k}", name=f"x{k}") for k in range(KT)]
    wqk = [sb.tile([128, 512], BF16, tag=f"wqk{k}", name=f"wqk{k}") for k in range(KT)]
    wv = [sb.tile([128, 256], BF16, tag=f"wv{k}", name=f"wv{k}") for k in range(KT)]
    wo = [sb.tile([128, 512], BF16, tag=f"wo{t}", name=f"wo{t}") for t in range(2)]
    qkT = [sb.tile([128, N], BF16, tag=f"qk{o}", name=f"qk{o}") for o in range(4)]
    vsb = [sb.tile([128, HPC * 65], BF16, tag=f"v{t}", name=f"v{t}") for t in range(NT)]
    AT = [sb.tile([128, N], BF16, tag=f"at{t}", name=f"at{t}") for t in range(2)]

    # ---- input DMAs, ordered by first use and spread over 4 engine queues
    # so the first qk-projection inputs land as early as possible ----
    dq = [nc.sync, nc.scalar, nc.gpsimd, nc.gpsimd]
    if not cfg["dma_spread"]:
        dq = [nc.sync, nc.scalar, nc.sync, nc.scalar]
    for k in range(KT):
        dq[k % 4].dma_start(out=wqk[k][:], in_=wqkT_d[k * 128:(k + 1) * 128, :])
    for k in range(KT):
        dq[(k + 2) % 4].dma_start(
            out=xT[k][:, 0:512], in_=xT_d[k * 128:(k + 1) * 128, 0:512])
    for nch in range(1, 4):
        for k in range(KT):
            dq[k % 2].dma_start(
                out=xT[k][:, nch * 512:(nch + 1) * 512],
                in_=xT_d[k * 128:(k + 1) * 128, nch * 512:(nch + 1) * 512],
            )
    for k in range(KT):
        dq[2 + k % 2].dma_start(out=wv[k][:], in_=wvT_d[k * 128:(k + 1) * 128, :])
    for t in range(2):
        dq[2 + t].dma_start(out=wo[t][:], in_=woT_d[t * 128:(t + 1) * 128, :])

    # HAM warm-up: ~4us of dummy matmuls while the input DMAs stream, so the
    # first real matmuls run at 2.4 GHz instead of the cold 1.2 GHz default.
    # The dummy memset comes FIRST on the DVE queue so the matmuls can start
    # ~0.3us in.
    def emit_warmup():
        dummy = sb.tile([128, 512], BF16, tag="dummy", name="dummy")
        nc.vector.memset(dummy[:], 1.0)
        dps = pproj.tile([128, PROJ_W], F32, tag=PROJ_TAG, name="warm")
        for i in range(cfg["n_warm"]):
            nc.tensor.matmul(dps[:, 0:512], dummy[:, 0:128], dummy[:],
                             start=True, stop=True)

    if cfg["warm_first"]:
        emit_warmup()
    # ones columns of v tiles (never overwritten by the v eviction)
    for t in range(NT):
        v3 = vsb[t][:].rearrange("p (h c) -> p h c", c=65)
        nc.vector.memset(v3[:, :, 64:65], 1.0)
    if not cfg["warm_first"]:
        emit_warmup()

    # ---- projection emitters (dripped into the attention pipeline) ----
    # qk o-tiles: 0 = q heads 0/1, 1 = q heads 2/3, 2 = k heads 0/1, 3 = k 2/3.
    def qk_chunk(ot, nch):
        def f():
            ps = pproj.tile([128, PROJ_W], F32, tag=PROJ_TAG, name="pp")
            for k in range(KT):
                nc.tensor.matmul(
                    ps[:, 0:512],
                    wqk[k][:, ot * 128:(ot + 1) * 128],
                    xT[k][:, nch * 512:(nch + 1) * 512],
                    start=(k == 0),
                    stop=(k == KT - 1),
                )
            nc.scalar.copy(qkT[ot][:, nch * 512:(nch + 1) * 512], ps[:, 0:512])
        return f

    def v_chunk(t):
        def f():
            ps = pproj.tile([128, PROJ_W], F32, tag=PROJ_TAG, name="pp")
            for k in range(KT):
                nc.tensor.matmul(
                    ps[:, 0:256],
                    xT[k][:, t * 128:(t + 1) * 128],
                    wv[k][:],
                    start=(k == 0),
                    stop=(k == KT - 1),
                )
            v3 = vsb[t][:].rearrange("p (h c) -> p h c", c=65)
            p3 = ps[:, 0:256].rearrange("p (h c) -> p h c", c=64)
            nc.vector.tensor_copy(v3[:, :, 0:64], p3)
        return f

    def proj_chunk(nt, evict_scalar=False):
        def f():
            pp = pproj.tile([128, PROJ_W], F32, tag=PROJ_TAG, name="pp")
            for t2 in range(2):
                nc.tensor.matmul(
                    pp[:, 0:512],
                    AT[t2][:, nt * 128:(nt + 1) * 128],
                    wo[t2][:],
                    start=(t2 == 0),
                    stop=(t2 == 1),
                )
            ot_s = work.tile([128, 512], BF16, tag="o", name="ot_s")
            if evict_scalar:
                # kernel tail: ScalarE is idle; keep DVE free for the norms
                nc.scalar.copy(ot_s[:], pp[:, 0:512])
            else:
                nc.vector.tensor_copy(ot_s[:], pp[:, 0:512])
            nc.sync.dma_start(out=out_d[nt * 128:(nt + 1) * 128, :], in_=ot_s[:])
        return f

    def emit_pv(hp, jt, pt, pu_e, pu_o):
        nc.tensor.matmul(
            pu_e[0:65, :], vsb[jt][:, (2 * hp) * 65:(2 * hp) * 65 + 65],
            pt[:, 0:512], start=(jt == 0), stop=(jt == NT - 1))
        nc.tensor.matmul(
            pu_o[0:65, :], vsb[jt][:, (2 * hp + 1) * 65:(2 * hp + 1) * 65 + 65],
            pt[:, 512:1024], start=(jt == 0), stop=(jt == NT - 1))

    def stage_u(h, pu):
        # Stage [U | den] to SBUF: releases the PSUM accumulator for the next
        # unit's PV chain, and gives reciprocal_approx_fast the SBUF,
        # partition-0 input it requires (its raw-bit seed is wrong on the
        # PSUM read path and at nonzero base partitions). In the superstep
        # config these copies ride ScalarE (DVE carries half the exp stream).
        usb = work.tile([65, 512], F32, tag=f"usb{h % 2}", name="usb")
        den = work.tile([1, 512], F32, tag=f"den{h % 2}", name="den")
        if cfg["superstep"]:
            nc.scalar.copy(usb[:], pu[0:65, :])
            nc.scalar.copy(den[:], usb[64:65, :])
        else:
            nc.vector.tensor_copy(usb[:], pu[0:65, :])
            nc.vector.tensor_copy(den[:], usb[64:65, :])
        return usb, den

    def norm_math(h, ib, usb, den, c0=0, c1=512):
        cs = slice(c0, c1)
        rc = work.tile([1, 512], F32, tag=f"rc{h % 2}", name="rc")
        nc.vector.reciprocal_approx_fast(out=rc[0:1, cs], in_=den[0:1, cs])
        rbc = work.tile([64, 512], F32, tag=f"rbc{h % 2}", name="rbc")
        nc.gpsimd.partition_broadcast(rbc[:, cs], rc[0:1, cs], channels=64)
        rows = slice((h % 2) * 64, (h % 2) * 64 + 64)
        nc.vector.tensor_tensor(
            out=AT[h // 2][rows, ib * 512 + c0:ib * 512 + c1],
            in0=usb[0:64, cs], in1=rbc[:, cs], op=mult_t)

    # ---- pipeline ----
    # Prologue: k/q tiles for heads 0/1 (chunk 0 asap), then drip the rest.
    qk_chunk(2, 0)()
    qk_chunk(0, 0)()
    # Filler order contract: unit 0 pops fillers[s] right before step s
    # (20 pops with the PV drain), so v_chunk(jt) must sit at index <= jt+4
    # (PV(jt) is emitted after the pop at step jt+LAG), and qk_chunk(2, c)
    # at index <= 4*c - 1 (scores step 4c reads it after the pop at 4c).
    fillers = [qk_chunk(2, 1), qk_chunk(2, 2), qk_chunk(2, 3), qk_chunk(0, 1)]
    fillers += [v_chunk(t) for t in range(NT)]
    fillers += [qk_chunk(0, 2), qk_chunk(0, 3)]
    fillers += [qk_chunk(1, 0), qk_chunk(1, 1), qk_chunk(1, 2), qk_chunk(1, 3)]
    fillers += [qk_chunk(3, 0), qk_chunk(3, 1), qk_chunk(3, 2), qk_chunk(3, 3)]

    # Global software pipeline across all 8 units: scores/exp for step g run
    # alongside PV for step g-LAG (possibly of the previous unit), so the PE
    # never drains at unit boundaries and HAM stays warm.
    units = [(hp, ib) for hp in range(2) for ib in range(IB)]
    LAG = cfg["lag"]
    total = len(units) * NT
    pts = {}
    pu_e = pu_o = None
    pss = {}

    def emit_scores(g):
        u, s = g // NT, g % NT
        hp, ib = units[u]
        q_t = qkT[hp]
        k_t = qkT[2 + hp]
        # scores pair: concurrent row groups, fixed bank parity
        ps = pexp.tile([128, 1024], F32, tag="ps", name="ps")
        nc.tensor.matmul(
            ps[:, 0:512], k_t[0:64, s * 128:(s + 1) * 128],
            q_t[0:64, ib * 512:(ib + 1) * 512], start=True, stop=True)
        nc.tensor.matmul(
            ps[:, 512:1024], k_t[64:128, s * 128:(s + 1) * 128],
            q_t[64:128, ib * 512:(ib + 1) * 512], start=True, stop=True)
        pss[g] = ps

    def emit_exp(g, on_dve):
        ps = pss.pop(g)
        pt = ptp.tile([128, 1024], BF16, tag=f"pt{g % 8}", name=f"pt{g % 8}")
        if on_dve:
            nc.vector.tensor_scalar(
                out=pt[:].bitcast(I16), in0=ps[:],
                scalar1=SCHR_A, scalar2=SCHR_B, op0=mult_t, op1=add_t)
        else:
            nc.scalar.activation(pt[:], ps[:], exp_t, scale=SCALE)
        pts[g] = pt

    def handle_pv(pg):
        nonlocal pu_e, pu_o
        u2, s2 = pg // NT, pg % NT
        hp2, ib2 = units[u2]
        if s2 == 0:
            pu_e = ppv.tile([128, 512], F32, tag="pu", name="pue")
            pu_o = ppv.tile([128, 512], F32, tag="pu", name="puo")
        emit_pv(hp2, s2, pts.pop(pg), pu_e, pu_o)
        if s2 == NT - 1:
            if cfg["tail_split"] and u2 == len(units) - 1:
                # Kernel tail: nothing reuses these PSUM banks, so skip the
                # SBUF staging, normalize straight out of PSUM in 128-col
                # quarters, and release each quarter's output-projection
                # chunk asap. The two den copies run on different engines.
                de = work.tile([1, 512], F32, tag="den0", name="den")
                nc.vector.tensor_copy(de[:], pu_e[64:65, :])
                do_ = work.tile([1, 512], F32, tag="den1", name="den")
                nc.scalar.copy(do_[:], pu_o[64:65, :])
                for c in range(4):
                    norm_math(2 * hp2, ib2, pu_e, de, c * 128, (c + 1) * 128)
                    norm_math(2 * hp2 + 1, ib2, pu_o, do_, c * 128, (c + 1) * 128)
                    fillers.append(proj_chunk(ib2 * 4 + c, evict_scalar=True))
            else:
                # release both PSUM accumulators first, then the math
                ue, de = stage_u(2 * hp2, pu_e)
                uo, do_ = stage_u(2 * hp2 + 1, pu_o)
                if hp2 == 1:
                    # interleave 256-col norm halves with their dependent
                    # output-projection chunks so they start ~1.3us earlier;
                    # evicts ride ScalarE's boundary lull (cfg "sevict")
                    for c in range(2):
                        norm_math(2 * hp2, ib2, ue, de, c * 256, (c + 1) * 256)
                        norm_math(2 * hp2 + 1, ib2, uo, do_,
                                  c * 256, (c + 1) * 256)
                        fillers.append(proj_chunk(
                            ib2 * 4 + 2 * c, evict_scalar=cfg["sevict"]))
                        fillers.append(proj_chunk(
                            ib2 * 4 + 2 * c + 1, evict_scalar=cfg["sevict"]))
                else:
                    norm_math(2 * hp2, ib2, ue, de)
                    norm_math(2 * hp2 + 1, ib2, uo, do_)

    if cfg["superstep"]:
        # two steps per iteration: 4 score MMs back-to-back (the rg0/rg64
        # pairs of consecutive steps overlap; the K=128 drain bubble is paid
        # once), exp(even)->ScalarE and exp(odd)->DVE run concurrently, then
        # the two lagged PV pairs.
        for G in range(0, total + LAG, 2):
            for g in (G, G + 1):
                if g < total:
                    if fillers:
                        fillers.pop(0)()
                    emit_scores(g)
            for g in (G, G + 1):
                if g < total:
                    emit_exp(g, on_dve=(g % 2 == 1))
            for g in (G - LAG, G + 1 - LAG):
                if 0 <= g < total:
                    handle_pv(g)
    else:
        for g in range(total + LAG):
            if g < total:
                u, s = g // NT, g % NT
                if fillers:
                    fillers.pop(0)()
                emit_scores(g)
                if cfg["f50"]:
                    on_dve = (s % 2 == 1)
                elif cfg["tail_split"] and u == len(units) - 1 and s >= 8:
                    on_dve = (s % 2 == 1)
                else:
                    on_dve = s in cfg["dve_steps"]
                emit_exp(g, on_dve)
            if g - LAG >= 0:
                handle_pv(g - LAG)
    for f in fillers:
        f()


@functools.lru_cache(maxsize=8)
def _build(variant="cur"):
    nc = bacc.Bacc("TRN2", target_bir_lowering=False, debug=False,
                   num_devices=N_CORES)
    xT_d = nc.dram_tensor("xT", [C, N], BF16, kind="ExternalInput").ap()
    wqkT_d = nc.dram_tensor("wqkT", [C, 512], BF16, kind="ExternalInput").ap()
    wvT_d = nc.dram_tensor("wvT", [C, 256], BF16, kind="ExternalInput").ap()
    woT_d = nc.dram_tensor("woT", [256, C], BF16, kind="ExternalInput").ap()
    out_d = nc.dram_tensor("out", [N, C], BF16, kind="ExternalOutput").ap()
    from contextlib import ExitStack
    with tile.TileContext(nc) as tc, ExitStack() as ctx:
        _build_body(nc, tc, ctx, xT_d, wqkT_d, wvT_d, woT_d, out_d,
                    CONFIGS[variant])
    nc.compile()
    return nc


def _shard_inputs(x, W_qkv, W_out):
    bf16 = ml_dtypes.bfloat16
    in_maps = []
    for core in range(N_CORES):
        b, hg = core // 2, core % 2
        xT = np.ascontiguousarray(x[b].T).astype(bf16)
        rows_q = W_qkv[hg * 256:(hg + 1) * 256, :]
        rows_k = W_qkv[512 + hg * 256:512 + (hg + 1) * 256, :]
        wqkT = np.ascontiguousarray(
            np.concatenate([rows_q, rows_k], 0).T).astype(bf16)
        wvT = np.ascontiguousarray(
            W_qkv[1024 + hg * 256:1024 + (hg + 1) * 256, :].T).astype(bf16)
        woT = np.ascontiguousarray(
            W_out[:, hg * 256:(hg + 1) * 256].T).astype(bf16)
        in_maps.append({"xT": xT, "wqkT": wqkT, "wvT": wvT, "woT": woT})
    return in_maps


def _run(inputs, trace=False, tmpdir=None, variant="cur"):
    x = np.asarray(inputs["x"], dtype=np.float32)
    W_qkv = np.asarray(inputs["W_qkv"], dtype=np.float32)
    W_out = np.asarray(inputs["W_out"], dtype=np.float32)
    nc = _build(variant)
    in_maps = _shard_inputs(x, W_qkv, W_out)
    kwargs = {}
    if trace:
        kwargs = dict(trace=True, tmpdir=tmpdir)
    res = run_bass_kernel_spmd(nc, in_maps, core_ids=list(range(N_CORES)), **kwargs)
    out = np.zeros((B, N, C), np.float32)
    for core in range(N_CORES):
        out[core // 2] += res.results[core]["out"].astype(np.float32)
    return out, res


def kernel(**inputs):
    out, _ = _run(inputs)
    return out
